# revision 14
# baseline (speedup 1.0000x reference)
"""Pairwise squared-euclidean distance kernel for Trainium2 (8 NeuronCores).

z[i, j] = ||x_i||^2 + ||y_j||^2 - 2 * <x_i, y_j>

Sharding: x rows split across 8 cores (1024 rows each), y replicated.
Each core computes a [1024, 8192] tile of the output with no communication.

Per-core algorithm (v6 — hybrid transpose, fp16 out, HAM-warm):
  1. x: fp32 load for row norms (ScalarE Square+accum); DVE casts
     x*(-2) to bf16; PE-transposed (16 tiles) during the pipeline head.
  2. y quarters 0-1: fp32 chunk loads -> DVE cast bf16 -> PE transpose
     (bf16, 1 cyc/col) -> PSUM -> evac to yT, interleaved with the
     first main groups so the PE never idles.
     y quarters 2-3: ScalarE-cast chunks bounce through DRAM
     (contiguous d-halves) and return via XBAR DMA-transpose - zero PE
     cost, using later-phase DMA slack. (fp32 XBAR is not supported by
     HW: wedges the device. SWDGE cast-DMA races its consumers, and
     gpsimd engine casts are 6x slower than DVE/ACT - avoid all.)
  3. yTsq = yT0^2 + yT1^2 (DVE, bf16, per chunk): ysq rides the PE as
     a third matmul pass per bank (all-ones stationary broadcasts
     ysq_j to every partition).
  4. Main loop: 32 groups (y quarter x m-tile) of 4 PSUM banks; per
     bank 3 passes (xT0, xT1, ones). Evac = one fused op per bank:
     ScalarE activation(psum + xsq) / DVE tensor_scalar(psum + xsq),
     alternating, straight to fp16. Host upcasts fp16 -> fp32.
  5. Output DMA alternates sync/scalar HWDGE queues; back-to-back PE
     groups keep the HAM clock gate warm (2.4 GHz).

Known-good environment notes: tensor_tensor_reduce crashes the device
(NRT_EXEC_UNIT_UNRECOVERABLE) - do not use. fp32r matmuls never warm
the HAM clock gate and self-load weights serially (~536ns/mm).
"""

import os

import numpy as np

import concourse.bacc as bacc
import concourse.mybir as mybir
import concourse.tile as tile
from concourse.bass_utils import run_bass_kernel_spmd
from concourse.masks import make_identity

N_CORES = 8
N_FULL = 8192  # total x rows
M_Y = 8192  # y rows
D = 256  # feature dim
N_SHARD = N_FULL // N_CORES  # 1024 x rows per core

P = 128
NT = 512  # matmul free-dim tile (one fp32 PSUM bank)
GRP = 4  # PSUM banks per group (one y quarter)
QCOLS = GRP * NT  # 2048
Q = M_Y // QCOLS  # 4 y quarters
M_TILES = N_SHARD // P  # 8
YCH = 1024  # y rows per load chunk

FP32 = mybir.dt.float32
BF16 = mybir.dt.bfloat16
FP16 = mybir.dt.float16
AF = mybir.ActivationFunctionType
ALU = mybir.AluOpType

_CACHE = {}
LAST_RESULTS = None


def _build():
    nc = bacc.Bacc("TRN2", target_bir_lowering=False, debug=False, num_devices=N_CORES)
    x_d = nc.dram_tensor("x", [N_SHARD, D], FP32, kind="ExternalInput").ap()
    y_d = nc.dram_tensor("y", [M_Y, D], FP32, kind="ExternalInput").ap()
    out_d = nc.dram_tensor("out", [N_SHARD, M_Y], FP16, kind="ExternalOutput").ap()

    with tile.TileContext(nc) as tc:
        with (
            tc.tile_pool(name="const", bufs=1) as const,
            tc.tile_pool(name="sq", bufs=4) as sqp,
            tc.tile_pool(name="ystage", bufs=5) as ystage,
            tc.tile_pool(name="outp", bufs=3) as outp,
            tc.tile_pool(name="dramp", bufs=1, space="DRAM") as dramp,
            tc.tile_pool(name="psmm", bufs=8, space="PSUM") as psmm,
        ):
            ones = const.tile([P, P], BF16)
            nc.vector.memset(ones[:], 1.0)
            identity = const.tile([P, P], BF16)
            make_identity(nc, identity)

            xsq = const.tile([P, M_TILES], FP32)
            x_nat = const.tile([P, M_TILES, D], FP32)
            xbf_sb = const.tile([P, M_TILES, D], BF16)
            xT = [const.tile([P, N_SHARD], BF16, name=f"xT{c}") for c in range(2)]
            yT = [const.tile([P, M_Y], BF16, name=f"yT{c}") for c in range(2)]
            yTsq = const.tile([P, M_Y], BF16, name="yTsq")

            ybf = [dramp.tile([M_Y, P], BF16, name=f"ybf{c}") for c in range(2)]

            # ---- x: load fp32, cast*(-2), PE transpose, row norms ----
            nc.sync.dma_start(x_nat[:], x_d.rearrange("(t p) d -> p t d", p=P))
            nc.vector.tensor_scalar_mul(xbf_sb[:], x_nat[:], -2.0)
            for c in range(2):
                ps = psmm.tile([P, N_SHARD], BF16, tag="mm", name=f"xtr{c}")
                for t in range(M_TILES):
                    nc.tensor.transpose(
                        ps[:, t * P : (t + 1) * P],
                        xbf_sb[:, t, c * P : (c + 1) * P],
                        identity,
                    )
                if c == 0:
                    nc.vector.tensor_copy(xT[c][:, :], ps[:])
                else:
                    nc.scalar.copy(xT[c][:, :], ps[:])
            for t in range(M_TILES):
                sq = sqp.tile([P, D], FP32, tag="sq")
                nc.scalar.activation(
                    sq[:], x_nat[:, t, :], AF.Square, accum_out=xsq[:, t : t + 1]
                )

            # ---- y chunk staging ----
            ycast = {}

            def y_load(ch, eng=None):
                rows = slice(ch * YCH, (ch + 1) * YCH)
                yst = ystage.tile([P, YCH // P, D], FP32, tag="yst")
                (eng or nc.sync).dma_start(
                    yst[:], y_d[rows, :].rearrange("(t p) d -> p t d", p=P)
                )
                ycast[ch] = yst

            def y_cast(ch, eng):
                ybf_sb = ystage.tile([P, YCH // P, D], BF16, tag="ybf_sb")
                if eng is nc.scalar:
                    eng.copy(ybf_sb[:], ycast[ch][:])
                else:
                    eng.tensor_copy(ybf_sb[:], ycast[ch][:])
                ycast[ch] = ybf_sb

            def y_store(ch):
                rows = slice(ch * YCH, (ch + 1) * YCH)
                for c in range(2):
                    nc.scalar.dma_start(
                        ybf[c][rows, :].rearrange("(t p) d -> p t d", p=P),
                        ycast[ch][:, :, c * P : (c + 1) * P],
                    )

            def y_xbar(q):
                rows = slice(q * QCOLS, (q + 1) * QCOLS)
                for c in range(2):
                    nc.sync.dma_start_transpose(yT[c][:, rows], ybf[c][rows, :])

            def y_petr_chunk(ch):
                src = ycast[ch]
                for c in range(2):
                    ps = psmm.tile([P, YCH], BF16, tag="mm", name=f"tr_{ch}_{c}")
                    for t in range(YCH // P):
                        nc.tensor.transpose(
                            ps[:, t * P : (t + 1) * P],
                            src[:, t, c * P : (c + 1) * P],
                            identity,
                        )
                    cols = slice(ch * YCH, (ch + 1) * YCH)
                    if c == 0:
                        nc.vector.tensor_copy(yT[c][:, cols], ps[:])
                    else:
                        nc.scalar.copy(yT[c][:, cols], ps[:])

            def ytsq_chunk(ch):
                cs = slice(ch * YCH, (ch + 1) * YCH)
                t0 = sqp.tile([P, YCH], BF16, tag="t0")
                nc.vector.tensor_tensor(t0[:], yT[0][:, cs], yT[0][:, cs], ALU.mult)
                t1 = sqp.tile([P, YCH], BF16, tag="t1")
                nc.vector.tensor_tensor(t1[:], yT[1][:, cs], yT[1][:, cs], ALU.mult)
                nc.gpsimd.tensor_tensor(yTsq[:, cs], t0[:], t1[:], ALU.add)

            # head: q0+q1 chunks load first; q0 PE-transposed immediately
            for ch in range(4):
                y_load(ch)
            y_cast(0, nc.vector)
            y_petr_chunk(0)
            ytsq_chunk(0)
            y_cast(1, nc.vector)
            y_petr_chunk(1)
            ytsq_chunk(1)

            # ---- main: per (quarter, m-tile): 4 banks x 3 passes ----
            for q in range(Q):
                for m in range(M_TILES):
                    # stream the rest of y behind the main loop
                    if q == 0 and m in (0, 1):
                        y_cast(2 + m, nc.vector)
                        y_petr_chunk(2 + m)  # q1 chunks on PE
                        ytsq_chunk(2 + m)
                    if q == 0 and m in (2, 3, 4, 5):
                        ch = 2 + m
                        y_load(ch, nc.scalar if m >= 4 else nc.sync)
                        y_cast(ch, nc.scalar if m % 2 == 0 else nc.vector)
                        y_store(ch)
                    if q == 0 and m == 4:
                        y_xbar(2)
                    if q == 0 and m == 6:
                        y_xbar(3)
                    if q == 1 and m in (2, 3):
                        ytsq_chunk(2 + m)  # chunks 4,5 (q2)
                    if q == 2 and m in (1, 2):
                        ytsq_chunk(5 + m)  # chunks 6,7 (q3)
                    lhs0 = xT[0][:, m * P : (m + 1) * P]
                    lhs1 = xT[1][:, m * P : (m + 1) * P]
                    ot = outp.tile([P, QCOLS], FP16, tag="ot")
                    pms = [
                        psmm.tile([P, NT], FP32, tag="mm", name=f"pm_{q}_{m}_{k}")
                        for k in range(GRP)
                    ]
                    for k in range(GRP):
                        n = q * GRP + k
                        nc.tensor.matmul(
                            pms[k][:], lhs0, yT[0][:, n * NT : (n + 1) * NT],
                            start=True, stop=False,
                        )
                    for k in range(GRP):
                        n = q * GRP + k
                        nc.tensor.matmul(
                            pms[k][:], lhs1, yT[1][:, n * NT : (n + 1) * NT],
                            start=False, stop=False,
                        )
                    for k in range(GRP):
                        n = q * GRP + k
                        nc.tensor.matmul(
                            pms[k][:], ones[:], yTsq[:, n * NT : (n + 1) * NT],
                            start=False, stop=True,
                        )
                    for k in range(GRP):
                        osl = ot[:, k * NT : (k + 1) * NT]
                        if k % 2 == 0:
                            nc.scalar.activation(
                                osl, pms[k][:], AF.Identity,
                                bias=xsq[:, m : m + 1], scale=1.0,
                            )
                        else:
                            nc.vector.tensor_scalar_add(
                                osl, pms[k][:], xsq[:, m : m + 1]
                            )
                    out_eng = nc.sync if m % 2 == 0 else nc.scalar
                    out_eng.dma_start(
                        out_d[m * P : (m + 1) * P, q * QCOLS : (q + 1) * QCOLS],
                        ot[:],
                    )

    nc.compile()
    return nc


def _get_nc():
    if "nc" not in _CACHE:
        _CACHE["nc"] = _build()
    return _CACHE["nc"]


def kernel(x: np.ndarray, y: np.ndarray) -> np.ndarray:
    global LAST_RESULTS
    x = np.ascontiguousarray(np.asarray(x, dtype=np.float32))
    y = np.ascontiguousarray(np.asarray(y, dtype=np.float32))
    assert x.shape == (N_FULL, D) and y.shape == (M_Y, D)

    nc = _get_nc()
    in_maps = [
        {"x": x[i * N_SHARD : (i + 1) * N_SHARD], "y": y} for i in range(N_CORES)
    ]
    res = run_bass_kernel_spmd(
        nc,
        in_maps,
        core_ids=list(range(N_CORES)),
        trace=bool(os.environ.get("BASS_KERNEL_TRACE")),
    )
    LAST_RESULTS = res
    return np.concatenate(
        [res.results[i]["out"].astype(np.float32) for i in range(N_CORES)], axis=0
    )


# revision 15
# speedup vs baseline: 1.0731x; 1.0731x over previous
"""Pairwise squared-euclidean distance kernel for Trainium2 (8 NeuronCores).

z[i, j] = ||x_i||^2 + ||y_j||^2 - 2 * <x_i, y_j>

Sharding: x rows split across 8 cores (1024 rows each), y replicated.
Each core computes a [1024, 8192] tile of the output with no communication.

Per-core algorithm (v6 — hybrid transpose, fp16 out, HAM-warm):
  1. x: fp32 load for row norms (ScalarE Square+accum); DVE casts
     x*(-2) to bf16; PE-transposed (16 tiles) during the pipeline head.
  2. y quarters 0-1: fp32 chunk loads -> DVE cast bf16 -> PE transpose
     (bf16, 1 cyc/col) -> PSUM -> evac to yT, interleaved with the
     first main groups so the PE never idles.
     y quarters 2-3: ScalarE-cast chunks bounce through DRAM
     (contiguous d-halves) and return via XBAR DMA-transpose - zero PE
     cost, using later-phase DMA slack. (fp32 XBAR is not supported by
     HW: wedges the device. SWDGE cast-DMA races its consumers, and
     gpsimd engine casts are 6x slower than DVE/ACT - avoid all.)
  3. yTsq = yT0^2 + yT1^2 (DVE, bf16, per chunk): ysq rides the PE as
     a third matmul pass per bank (all-ones stationary broadcasts
     ysq_j to every partition).
  4. Main loop: 32 groups (y quarter x m-tile) of 4 PSUM banks; per
     bank 3 passes (xT0, xT1, ones). Evac = one fused op per bank:
     ScalarE activation(psum + xsq) / DVE tensor_scalar(psum + xsq),
     alternating, straight to fp16. Host upcasts fp16 -> fp32.
  5. Output DMA alternates sync/scalar HWDGE queues; back-to-back PE
     groups keep the HAM clock gate warm (2.4 GHz).

Known-good environment notes: tensor_tensor_reduce crashes the device
(NRT_EXEC_UNIT_UNRECOVERABLE) - do not use. fp32r matmuls never warm
the HAM clock gate and self-load weights serially (~536ns/mm).
"""

import os

import numpy as np

import concourse.bacc as bacc
import concourse.mybir as mybir
import concourse.tile as tile
from concourse.bass_utils import run_bass_kernel_spmd
from concourse.masks import make_identity

N_CORES = 8
N_FULL = 8192  # total x rows
M_Y = 8192  # y rows
D = 256  # feature dim
N_SHARD = N_FULL // N_CORES  # 1024 x rows per core

P = 128
NT = 512  # matmul free-dim tile (one fp32 PSUM bank)
GRP = 4  # PSUM banks per group (one y quarter)
QCOLS = GRP * NT  # 2048
Q = M_Y // QCOLS  # 4 y quarters
M_TILES = N_SHARD // P  # 8
YCH = 1024  # y rows per load chunk

FP32 = mybir.dt.float32
BF16 = mybir.dt.bfloat16
FP16 = mybir.dt.float16
AF = mybir.ActivationFunctionType
ALU = mybir.AluOpType

_CACHE = {}
LAST_RESULTS = None


def _build():
    nc = bacc.Bacc("TRN2", target_bir_lowering=False, debug=False, num_devices=N_CORES)
    x_d = nc.dram_tensor("x", [N_SHARD, D], FP32, kind="ExternalInput").ap()
    y_d = nc.dram_tensor("y", [M_Y, D], FP32, kind="ExternalInput").ap()
    out_d = nc.dram_tensor("out", [N_SHARD, M_Y], FP16, kind="ExternalOutput").ap()

    with tile.TileContext(nc) as tc:
        with (
            tc.tile_pool(name="const", bufs=1) as const,
            tc.tile_pool(name="sq", bufs=4) as sqp,
            tc.tile_pool(name="ystage", bufs=5) as ystage,
            tc.tile_pool(name="outp", bufs=3) as outp,
            tc.tile_pool(name="dramp", bufs=1, space="DRAM") as dramp,
            tc.tile_pool(name="psmm", bufs=8, space="PSUM") as psmm,
        ):
            ones = const.tile([P, P], BF16)
            nc.vector.memset(ones[:], 1.0)
            identity = const.tile([P, P], BF16)
            make_identity(nc, identity)

            xsq = const.tile([P, M_TILES], FP32)
            x_nat = const.tile([P, M_TILES, D], FP32)
            xbf_sb = const.tile([P, M_TILES, D], BF16)
            xT = [const.tile([P, N_SHARD], BF16, name=f"xT{c}") for c in range(2)]
            yT = [const.tile([P, M_Y], BF16, name=f"yT{c}") for c in range(2)]
            yTsq = const.tile([P, M_Y], BF16, name="yTsq")

            ybf = [dramp.tile([M_Y, P], BF16, name=f"ybf{c}") for c in range(2)]

            # ---- x: load fp32, cast*(-2), PE transpose, row norms ----
            nc.sync.dma_start(x_nat[:], x_d.rearrange("(t p) d -> p t d", p=P))
            nc.vector.tensor_scalar_mul(xbf_sb[:], x_nat[:], -2.0)
            for c in range(2):
                ps = psmm.tile([P, N_SHARD], BF16, tag="mm", name=f"xtr{c}")
                for t in range(M_TILES):
                    nc.tensor.transpose(
                        ps[:, t * P : (t + 1) * P],
                        xbf_sb[:, t, c * P : (c + 1) * P],
                        identity,
                    )
                if c == 0:
                    nc.vector.tensor_copy(xT[c][:, :], ps[:])
                else:
                    nc.scalar.copy(xT[c][:, :], ps[:])
            for t in range(M_TILES):
                sq = sqp.tile([P, D], FP32, tag="sq")
                nc.scalar.activation(
                    sq[:], x_nat[:, t, :], AF.Square, accum_out=xsq[:, t : t + 1]
                )

            # ---- y chunk staging ----
            ycast = {}

            def y_load(ch, eng=None):
                rows = slice(ch * YCH, (ch + 1) * YCH)
                yst = ystage.tile([P, YCH // P, D], FP32, tag="yst")
                (eng or nc.sync).dma_start(
                    yst[:], y_d[rows, :].rearrange("(t p) d -> p t d", p=P)
                )
                ycast[ch] = yst

            def y_cast(ch, eng):
                ybf_sb = ystage.tile([P, YCH // P, D], BF16, tag="ybf_sb")
                if eng is nc.scalar:
                    eng.copy(ybf_sb[:], ycast[ch][:])
                else:
                    eng.tensor_copy(ybf_sb[:], ycast[ch][:])
                ycast[ch] = ybf_sb

            def y_store(ch):
                rows = slice(ch * YCH, (ch + 1) * YCH)
                for c in range(2):
                    nc.sync.dma_start(
                        ybf[c][rows, :].rearrange("(t p) d -> p t d", p=P),
                        ycast[ch][:, :, c * P : (c + 1) * P],
                    )

            def y_xbar(q):
                rows = slice(q * QCOLS, (q + 1) * QCOLS)
                for c in range(2):
                    nc.sync.dma_start_transpose(yT[c][:, rows], ybf[c][rows, :])

            def y_petr_chunk(ch):
                src = ycast[ch]
                for c in range(2):
                    ps = psmm.tile([P, YCH], BF16, tag="mm", name=f"tr_{ch}_{c}")
                    for t in range(YCH // P):
                        nc.tensor.transpose(
                            ps[:, t * P : (t + 1) * P],
                            src[:, t, c * P : (c + 1) * P],
                            identity,
                        )
                    cols = slice(ch * YCH, (ch + 1) * YCH)
                    if c == 0:
                        nc.vector.tensor_copy(yT[c][:, cols], ps[:])
                    else:
                        nc.scalar.copy(yT[c][:, cols], ps[:])

            def ytsq_chunk(ch):
                cs = slice(ch * YCH, (ch + 1) * YCH)
                t0 = sqp.tile([P, YCH], BF16, tag="t0")
                nc.vector.tensor_tensor(t0[:], yT[0][:, cs], yT[0][:, cs], ALU.mult)
                t1 = sqp.tile([P, YCH], BF16, tag="t1")
                nc.vector.tensor_tensor(t1[:], yT[1][:, cs], yT[1][:, cs], ALU.mult)
                nc.vector.tensor_tensor(yTsq[:, cs], t0[:], t1[:], ALU.add)

            # head: q0+q1 chunks load first; q0 PE-transposed immediately
            for ch in range(4):
                y_load(ch)
            y_cast(0, nc.vector)
            y_petr_chunk(0)
            ytsq_chunk(0)
            y_cast(1, nc.vector)
            y_petr_chunk(1)
            ytsq_chunk(1)

            # ---- main: per (quarter, m-tile): 4 banks x 3 passes ----
            for q in range(Q):
                for m in range(M_TILES):
                    # stream the rest of y behind the main loop
                    if q == 0 and m in (0, 1):
                        y_cast(2 + m, nc.vector)
                        y_petr_chunk(2 + m)  # q1 chunks on PE
                        ytsq_chunk(2 + m)
                    if q == 0 and m in (2, 3, 4, 5):
                        ch = 2 + m
                        y_load(ch, nc.scalar if m >= 4 else nc.sync)
                        y_cast(ch, nc.scalar)
                        y_store(ch)
                    if q == 0 and m == 4:
                        y_xbar(2)
                    if q == 0 and m == 6:
                        y_xbar(3)
                    if q == 1 and m in (2, 3):
                        ytsq_chunk(2 + m)  # chunks 4,5 (q2)
                    if q == 2 and m in (1, 2):
                        ytsq_chunk(5 + m)  # chunks 6,7 (q3)
                    lhs0 = xT[0][:, m * P : (m + 1) * P]
                    lhs1 = xT[1][:, m * P : (m + 1) * P]
                    ot = outp.tile([P, QCOLS], FP16, tag="ot")
                    pms = [
                        psmm.tile([P, NT], FP32, tag="mm", name=f"pm_{q}_{m}_{k}")
                        for k in range(GRP)
                    ]
                    for k in range(GRP):
                        n = q * GRP + k
                        nc.tensor.matmul(
                            pms[k][:], lhs0, yT[0][:, n * NT : (n + 1) * NT],
                            start=True, stop=False,
                        )
                    for k in range(GRP):
                        n = q * GRP + k
                        nc.tensor.matmul(
                            pms[k][:], lhs1, yT[1][:, n * NT : (n + 1) * NT],
                            start=False, stop=False,
                        )
                    for k in range(GRP):
                        n = q * GRP + k
                        nc.tensor.matmul(
                            pms[k][:], ones[:], yTsq[:, n * NT : (n + 1) * NT],
                            start=False, stop=True,
                        )
                    for k in range(GRP):
                        osl = ot[:, k * NT : (k + 1) * NT]
                        if k % 2 == 0:
                            nc.scalar.activation(
                                osl, pms[k][:], AF.Identity,
                                bias=xsq[:, m : m + 1], scale=1.0,
                            )
                        else:
                            nc.vector.tensor_scalar_add(
                                osl, pms[k][:], xsq[:, m : m + 1]
                            )
                    out_eng = nc.sync if m % 2 == 0 else nc.scalar
                    out_eng.dma_start(
                        out_d[m * P : (m + 1) * P, q * QCOLS : (q + 1) * QCOLS],
                        ot[:],
                    )

    nc.compile()
    return nc


def _get_nc():
    if "nc" not in _CACHE:
        _CACHE["nc"] = _build()
    return _CACHE["nc"]


def kernel(x: np.ndarray, y: np.ndarray) -> np.ndarray:
    global LAST_RESULTS
    x = np.ascontiguousarray(np.asarray(x, dtype=np.float32))
    y = np.ascontiguousarray(np.asarray(y, dtype=np.float32))
    assert x.shape == (N_FULL, D) and y.shape == (M_Y, D)

    nc = _get_nc()
    in_maps = [
        {"x": x[i * N_SHARD : (i + 1) * N_SHARD], "y": y} for i in range(N_CORES)
    ]
    res = run_bass_kernel_spmd(
        nc,
        in_maps,
        core_ids=list(range(N_CORES)),
        trace=bool(os.environ.get("BASS_KERNEL_TRACE")),
    )
    LAST_RESULTS = res
    return np.concatenate(
        [res.results[i]["out"].astype(np.float32) for i in range(N_CORES)], axis=0
    )
